# revision 35
# baseline (speedup 1.0000x reference)
"""CrossNetMix (DCN-V2 mixture-of-low-rank-experts) Trainium2 kernel.

Data-parallel over batch across 8 cores (2048 rows each); feature-major
([d, b]) on chip so every matmul contraction lands on SBUF partitions.

Matmul stages (gating, V, C, U) run in fp8-e4m3 DoubleRow mode: each
matmul contracts two adjacent 128-row k-subtiles per pass (2 rows/cycle),
roughly halving tensor-engine time vs bf16. PSUM accumulates in fp32.
The softmax/gate-broadcast helper matmuls stay bf16.

Residual reformulation: with S_i = sum_{j<i} (uv_j + b_j),
  xi_i = x0 ⊙ (S_i + 1) .
Each layer's U matmuls accumulate uv into PSUM; for layers > 0 an identity
matmul adds the previous S (bf16, SBUF) into the same accumulation. Then
one scalar_tensor_tensor per d-chunk emits xi = (S + (1 + B_i)) ⊙ x0
directly in fp8 for the next layer's matmuls (bf16 on the last layer for
the output DMA), with the bias cumsum B_i folded into the per-partition
scalar. An ACT copy spills S back to SBUF for the next layer. This keeps
the whole combine at ~2 elementwise ops per d-chunk with no bf16 residual
round-trip on x0.

Chunks are processed in interleaved pairs (A, B): per layer the stream is
A.gating, A.V, A.C, A.U+combine then B.*, so B's matmuls cover A's
combine/softmax tails. Input DMAs are split across both HWDGE queues
(x via Sync/q1, V weights via Scalar/q10) in need-order.
"""

import os
import sys

import ml_dtypes
import numpy as np

if "/opt/trn_rl_repo" not in sys.path:
    sys.path.insert(0, "/opt/trn_rl_repo")

import concourse.bass as bass
import concourse.bacc as bacc
import concourse.bass_isa as bass_isa
import concourse.mybir as mybir
from concourse.tile import TileContext
from concourse.bass_utils import run_bass_kernel_spmd

AF = mybir.ActivationFunctionType
OP = mybir.AluOpType
DR = mybir.MatmulPerfMode.DoubleRow
F32 = mybir.dt.float32
WDT = mybir.dt.bfloat16
F8 = mybir.dt.float8e4
BF16 = ml_dtypes.bfloat16
NPF8 = ml_dtypes.float8_e4m3

N_CROSS = 3
E = 8            # experts
D = 1024         # feature dim
R = 64           # low rank
B = 16384        # full batch
NCORES = 8
BC = B // NCORES  # rows per core
CHUNK = 512       # batch tile (matmul free dim)
NCHUNK = BC // CHUNK
P = 128
KC = D // P       # d-chunks
ER = E * R        # 512
MC = ER // P      # (e,r)-chunks


def _build():
    nc = bacc.Bacc(None)
    xT = nc.declare_dram_parameter("xT", [D, BC], WDT, isOutput=False)
    xT8 = nc.declare_dram_parameter("xT8", [D, BC], F8, isOutput=False)
    Vl = nc.declare_dram_parameter("Vl", [N_CROSS, D, ER], F8, isOutput=False)
    Cb = nc.declare_dram_parameter("Cb", [N_CROSS, MC, 2, P, P], F8, isOutput=False)
    Ul = nc.declare_dram_parameter("Ul", [N_CROSS, ER, D], F8, isOutput=False)
    # gating weights padded to 16 cols: DoubleRow lhsT outer step must be
    # a multiple of 16 (s3_lw dual-fp8 restriction)
    WgT = nc.declare_dram_parameter("WgT", [D, 2 * E], F8, isOutput=False)
    # bTc[i, p, kc] = 1 + sum_{j<=i} b[j, kc*P+p]  (per-partition stt scalar)
    bTc = nc.declare_dram_parameter("bTc", [N_CROSS, P, KC], F32, isOutput=False)
    sel = nc.declare_dram_parameter("sel", [E, MC + 1, P], WDT, isOutput=False)
    id128 = nc.declare_dram_parameter("id128", [P, P], WDT, isOutput=False)
    outT = nc.declare_dram_parameter("outT", [D, BC], WDT, isOutput=True)

    with TileContext(nc) as tc:
        with (
            tc.sbuf_pool(name="wpool", bufs=1) as wpool,
            tc.sbuf_pool(name="xpool", bufs=4) as xpool,
            tc.sbuf_pool(name="x8pool", bufs=3) as x8pool,
            tc.sbuf_pool(name="xipool", bufs=4) as xipool,
            tc.sbuf_pool(name="spool2", bufs=4) as spool2,
            tc.sbuf_pool(name="xopool", bufs=2) as xopool,
            tc.sbuf_pool(name="h1pool", bufs=2) as h1pool,
            tc.sbuf_pool(name="h2pool", bufs=2) as h2pool,
            tc.sbuf_pool(name="ypool", bufs=2) as ypool,
            tc.sbuf_pool(name="spool", bufs=2) as spool,
            tc.psum_pool(name="psmm", bufs=3) as psmm,
            tc.psum_pool(name="psu", bufs=2) as psu,
            tc.psum_pool(name="pswb", bufs=2) as pswb,
            tc.psum_pool(name="psg", bufs=1) as psg,
        ):
            xTr = xT.rearrange("(kc p) b -> p kc b", p=P)
            xT8r = xT8.rearrange("(kc p) b -> p kc b", p=P)
            outr = outT.rearrange("(kc p) b -> p kc b", p=P)
            Vlr = Vl.rearrange("i (kc p) m -> p i kc m", p=P)
            Ulr = Ul.rearrange("i (mc p) d -> p i mc d", p=P)
            Cbr = Cb.rearrange("i m j p s -> p i m j s")

            def alloc_x0(c):
                t = xpool.tile([P, KC, CHUNK], WDT, tag="x0", name=f"x0_{c}")
                t8 = x8pool.tile([P, KC, CHUNK], F8, tag="x08", name=f"x08_{c}")
                return t, t8

            def load_x8(tt, c):
                cbs = slice(c * CHUNK, (c + 1) * CHUNK)
                nc.sync.dma_start(tt[1], xT8r[:, :, cbs])

            def load_xbf(tt, c):
                cbs = slice(c * CHUNK, (c + 1) * CHUNK)
                nc.sync.dma_start(tt[0], xTr[:, :, cbs])

            def load_x0(c):
                tt = alloc_x0(c)
                load_x8(tt, c)
                load_xbf(tt, c)
                return tt

            wg_sb = wpool.tile([P, KC, 2 * E], F8)
            nc.scalar.dma_start(wg_sb, WgT.rearrange("(kc p) e -> p kc e", p=P))

            v_sb = wpool.tile([P, N_CROSS, KC, ER], F8)
            u_sb = wpool.tile([P, N_CROSS, MC, D], F8)
            c_sb = wpool.tile([P, N_CROSS, MC, 2, P], F8)
            b_sb = wpool.tile([P, N_CROSS, KC], F32)
            id_sb = wpool.tile([P, P], WDT)

            nc.scalar.dma_start(v_sb[:, 0, 0:KC // 2], Vlr[:, 0, 0:KC // 2])
            nc.scalar.dma_start(v_sb[:, 0, KC // 2:], Vlr[:, 0, KC // 2:])
            nc.scalar.dma_start(c_sb[:, 0], Cbr[:, 0])

            # q1 in need-order: x0c0 fp8 (gating), U0, x0c1 fp8 (B gating),
            # x0 bf16 copies (first needed at the combine), U1
            x0_tiles = {0: alloc_x0(0), 1: alloc_x0(1)}
            load_x8(x0_tiles[0], 0)
            nc.sync.dma_start(u_sb[:, 0], Ulr[:, 0])
            nc.sync.dma_start(id_sb, id128[:])
            load_x8(x0_tiles[1], 1)
            load_xbf(x0_tiles[0], 0)
            sel_sb = wpool.tile([E, MC + 1, P], WDT)
            nc.scalar.dma_start(sel_sb, sel[:])
            nc.scalar.dma_start(c_sb[:, 1:], Cbr[:, 1:])
            nc.scalar.dma_start(v_sb[:, 1], Vlr[:, 1])
            load_xbf(x0_tiles[1], 1)
            nc.sync.dma_start(u_sb[:, 1], Ulr[:, 1])
            nc.scalar.dma_start(b_sb, bTc.rearrange("i p kc -> p i kc"))
            nc.scalar.dma_start(v_sb[:, 2], Vlr[:, 2])
            nc.scalar.dma_start(u_sb[:, 2], Ulr[:, 2])

            s_tiles = {}
            xi8_tiles = {}
            for pair in range(NCHUNK // 2):
                for i in range(N_CROSS):
                    for half in range(2):
                        c = 2 * pair + half
                        if i == 1 and c + 2 < NCHUNK:
                            x0_tiles[c + 2] = load_x0(c + 2)
                        bs = slice(c * CHUNK, (c + 1) * CHUNK)
                        x0, x08 = x0_tiles[c]
                        src8 = x08 if i == 0 else xi8_tiles.pop(c)
                        s_prev = None if i == 0 else s_tiles.pop(c)
                        last = i == N_CROSS - 1
                        if last:
                            xi = xopool.tile([P, KC, CHUNK], WDT, tag="xiout")
                            x0_tiles.pop(c)
                        else:
                            xi = xipool.tile([P, KC, CHUNK], F8, tag="xi8")
                            xi8_tiles[c] = xi
                            s_new = spool2.tile([P, KC, CHUNK], WDT, tag="s")
                            s_tiles[c] = s_new
                        # ---- gating (fp8 DoubleRow, 16-col padded) ----
                        gps = psg.tile([2 * E, CHUNK], F32, tag="g")
                        for q in range(KC // 2):
                            nc.tensor.matmul(
                                gps,
                                wg_sb[:, 2 * q : 2 * q + 2, :],
                                src8[:, 2 * q : 2 * q + 2, :],
                                start=(q == 0),
                                stop=(q == KC // 2 - 1),
                                perf_mode=DR,
                            )
                        expg = spool.tile([E, CHUNK], WDT, tag="expg")
                        nc.scalar.activation(expg, gps[0:E, :], AF.Exp)
                        # ---- V stage (fp8 DoubleRow) ----
                        h1 = h1pool.tile([P, MC, CHUNK], F8, tag="h1")
                        for mc in range(MC):
                            vps = psmm.tile([P, CHUNK], F32, tag="mm")
                            for q in range(KC // 2):
                                nc.tensor.matmul(
                                    vps,
                                    v_sb[:, i, 2 * q : 2 * q + 2,
                                         mc * P : (mc + 1) * P],
                                    src8[:, 2 * q : 2 * q + 2, :],
                                    start=(q == 0),
                                    stop=(q == KC // 2 - 1),
                                    perf_mode=DR,
                                )
                            nc.scalar.activation(h1[:, mc, :], vps, AF.Tanh)
                        # ---- softmax tail: partition-sum on the idle
                        # gpsimd so the gate chain never queues behind the
                        # PE's V matmuls ----
                        allr = spool.tile([E, CHUNK], F32, tag="allr")
                        nc.gpsimd.partition_all_reduce(
                            allr, expg, channels=E,
                            reduce_op=bass_isa.ReduceOp.add,
                        )
                        rec8 = spool.tile([E, CHUNK], F32, tag="rfast")
                        nc.vector.reciprocal_approx_fast(rec8, allr)
                        wsb = spool.tile([E, CHUNK], WDT, tag="wsb")
                        nc.vector.tensor_tensor(wsb, expg, rec8, OP.mult)
                        # ---- C stage (fp8 DoubleRow, zero-padded pair) ----
                        ys = ypool.tile([P, MC, CHUNK], F8, tag="y")
                        for mc in range(MC):
                            cps = psmm.tile([P, CHUNK], F32, tag="mm")
                            qb = (mc // 2) * 2
                            nc.tensor.matmul(
                                cps,
                                c_sb[:, i, mc, :, :],
                                h1[:, qb : qb + 2, :],
                                start=True,
                                stop=True,
                                perf_mode=DR,
                            )
                            wbp = pswb.tile([P, CHUNK], F32, tag="wb")
                            nc.tensor.matmul(
                                wbp, sel_sb[:, mc, :], wsb, start=True, stop=True
                            )
                            h2 = h2pool.tile([P, CHUNK], F32, tag="h2")
                            nc.scalar.activation(h2, cps, AF.Tanh)
                            nc.vector.tensor_tensor(ys[:, mc, :], h2, wbp, OP.mult)
                        # ---- U stage + S accumulate + combine ----
                        for dc in range(KC):
                            ups = psu.tile([P, CHUNK], F32, tag="u")
                            for q in range(MC // 2):
                                nc.tensor.matmul(
                                    ups,
                                    u_sb[:, i, 2 * q : 2 * q + 2,
                                         dc * P : (dc + 1) * P],
                                    ys[:, 2 * q : 2 * q + 2, :],
                                    start=(q == 0),
                                    stop=(s_prev is None and q == MC // 2 - 1),
                                    perf_mode=DR,
                                )
                            if s_prev is not None:
                                nc.tensor.matmul(
                                    ups, id_sb, s_prev[:, dc, :],
                                    start=False, stop=True,
                                )
                            # xi = (S + (1 + B_i)) * x0
                            nc.vector.scalar_tensor_tensor(
                                xi[:, dc, :],
                                ups,
                                b_sb[:, i, dc : dc + 1],
                                x0[:, dc, :],
                                OP.add,
                                OP.mult,
                            )
                            if not last:
                                nc.scalar.activation(
                                    s_new[:, dc, :], ups, AF.Copy
                                )
                            else:
                                eng = nc.sync if dc % 2 == 0 else nc.scalar
                                eng.dma_start(outr[:, dc, bs], xi[:, dc, :])
    nc.compile()
    return nc


_CTX = {}


def _get_nc():
    if "nc" not in _CTX:
        _CTX["nc"] = _build()
    return _CTX["nc"]


def _prep_weights(U, V, C, Wg, b):
    f = np.float32
    U = np.asarray(U, dtype=f)
    V = np.asarray(V, dtype=f)
    C = np.asarray(C, dtype=f)
    Wg = np.asarray(Wg, dtype=f)
    b = np.asarray(b, dtype=f)
    # Vl[i, d, e*R+r] = V[i, e, d, r]
    Vl = np.ascontiguousarray(V.transpose(0, 2, 1, 3).reshape(N_CROSS, D, ER))
    # Ul[i, e*R+r, d] = U[i, e, d, r]
    Ul = np.ascontiguousarray(U.transpose(0, 1, 3, 2).reshape(N_CROSS, ER, D))
    # DoubleRow C: out-block mc pairs rhs h1 blocks (qb, qb+1); the plane
    # matching block mc carries the block-diag expert pair, the other is 0.
    Cb2 = np.zeros((N_CROSS, MC, 2, P, P), dtype=f)
    for i in range(N_CROSS):
        for m in range(MC):
            blk = np.zeros((P, P), dtype=f)
            blk[:R, :R] = C[i, 2 * m]
            blk[R:, R:] = C[i, 2 * m + 1]
            Cb2[i, m, m % 2] = blk
    WgT = np.zeros((D, 2 * E), dtype=f)
    WgT[:, :E] = Wg.T
    # bTc[i, p, kc] = 1 + cumsum_i b  (stt per-partition scalar)
    bc = 1.0 + np.cumsum(b, axis=0)
    bTc = np.ascontiguousarray(bc.reshape(N_CROSS, KC, P).transpose(0, 2, 1))
    sel = np.zeros((E, MC + 1, P), dtype=f)
    for m in range(MC):
        for j in range(P):
            sel[2 * m + j // R, m, j] = 1.0
    sel[:, MC, :] = 1.0
    return dict(
        Vl=Vl.astype(NPF8),
        Ul=Ul.astype(NPF8),
        Cb=Cb2.astype(NPF8),
        WgT=WgT.astype(NPF8),
        bTc=bTc,
        sel=sel.astype(BF16),
        id128=np.eye(P, dtype=f).astype(BF16),
    )


def kernel(x, U, V, C, Wg, b, _trace=False):
    nc = _get_nc()
    w = _prep_weights(U, V, C, Wg, b)
    xs = np.asarray(x, dtype=np.float32).reshape(NCORES, BC, D)
    in_maps = []
    for ci in range(NCORES):
        xt = np.ascontiguousarray(xs[ci].T)
        m = {"xT": xt.astype(BF16), "xT8": xt.astype(NPF8)}
        m.update(w)
        in_maps.append(m)
    res = run_bass_kernel_spmd(nc, in_maps, list(range(NCORES)), trace=_trace)
    kernel.last_result = res
    out = np.concatenate(
        [np.asarray(res.results[ci]["outT"]).astype(np.float32).T
         for ci in range(NCORES)],
        axis=0,
    )
    return np.ascontiguousarray(out, dtype=np.float32)


# revision 37
# speedup vs baseline: 1.0701x; 1.0701x over previous
"""CrossNetMix (DCN-V2 mixture-of-low-rank-experts) Trainium2 kernel.

Data-parallel over batch across 8 cores (2048 rows each); feature-major
([d, b]) on chip so every matmul contraction lands on SBUF partitions.

Matmul stages (gating, V, C, U) run in fp8-e4m3 DoubleRow mode: each
matmul contracts two adjacent 128-row k-subtiles per pass (2 rows/cycle),
roughly halving tensor-engine time vs bf16. PSUM accumulates in fp32.
The softmax/gate-broadcast helper matmuls stay bf16.

Residual reformulation: with S_i = sum_{j<i} (uv_j + b_j),
  xi_i = x0 ⊙ (S_i + 1) .
Each layer's U matmuls accumulate uv into PSUM; for layers > 0 an identity
matmul adds the previous S (bf16, SBUF) into the same accumulation. Then
one scalar_tensor_tensor per d-chunk emits xi = (S + (1 + B_i)) ⊙ x0
directly in fp8 for the next layer's matmuls (bf16 on the last layer for
the output DMA), with the bias cumsum B_i folded into the per-partition
scalar. An ACT copy spills S back to SBUF for the next layer. This keeps
the whole combine at ~2 elementwise ops per d-chunk with no bf16 residual
round-trip on x0.

Chunks are processed in interleaved pairs (A, B): per layer the stream is
A.gating, A.V, A.C, A.U+combine then B.*, so B's matmuls cover A's
combine/softmax tails. Input DMAs are split across both HWDGE queues
(x via Sync/q1, V weights via Scalar/q10) in need-order.
"""

import os
import sys

import ml_dtypes
import numpy as np

if "/opt/trn_rl_repo" not in sys.path:
    sys.path.insert(0, "/opt/trn_rl_repo")

import concourse.bass as bass
import concourse.bacc as bacc
import concourse.mybir as mybir
from concourse.tile import TileContext
from concourse.bass_utils import run_bass_kernel_spmd

AF = mybir.ActivationFunctionType
OP = mybir.AluOpType
DR = mybir.MatmulPerfMode.DoubleRow
F32 = mybir.dt.float32
WDT = mybir.dt.bfloat16
F8 = mybir.dt.float8e4
BF16 = ml_dtypes.bfloat16
NPF8 = ml_dtypes.float8_e4m3

N_CROSS = 3
E = 8            # experts
D = 1024         # feature dim
R = 64           # low rank
B = 16384        # full batch
NCORES = 8
BC = B // NCORES  # rows per core
CHUNK = 512       # batch tile (matmul free dim)
NCHUNK = BC // CHUNK
P = 128
KC = D // P       # d-chunks
ER = E * R        # 512
MC = ER // P      # (e,r)-chunks


def _build():
    nc = bacc.Bacc(None)
    xT = nc.declare_dram_parameter("xT", [D, BC], WDT, isOutput=False)
    xT8 = nc.declare_dram_parameter("xT8", [D, BC], F8, isOutput=False)
    Vl = nc.declare_dram_parameter("Vl", [N_CROSS, D, ER], F8, isOutput=False)
    Cb = nc.declare_dram_parameter("Cb", [N_CROSS, MC, 2, P, P], F8, isOutput=False)
    Ul = nc.declare_dram_parameter("Ul", [N_CROSS, ER, D], F8, isOutput=False)
    # gating weights padded to 16 cols: DoubleRow lhsT outer step must be
    # a multiple of 16 (s3_lw dual-fp8 restriction)
    WgT = nc.declare_dram_parameter("WgT", [D, 2 * E], F8, isOutput=False)
    # bTc[i, p, kc] = 1 + sum_{j<=i} b[j, kc*P+p]  (per-partition stt scalar)
    bTc = nc.declare_dram_parameter("bTc", [N_CROSS, P, KC], F32, isOutput=False)
    sel = nc.declare_dram_parameter("sel", [E, MC + 1, P], WDT, isOutput=False)
    id128 = nc.declare_dram_parameter("id128", [P, P], WDT, isOutput=False)
    outT = nc.declare_dram_parameter("outT", [D, BC], WDT, isOutput=True)

    with TileContext(nc) as tc:
        with (
            tc.sbuf_pool(name="wpool", bufs=1) as wpool,
            tc.sbuf_pool(name="xpool", bufs=4) as xpool,
            tc.sbuf_pool(name="x8pool", bufs=3) as x8pool,
            tc.sbuf_pool(name="xipool", bufs=4) as xipool,
            tc.sbuf_pool(name="spool2", bufs=4) as spool2,
            tc.sbuf_pool(name="xopool", bufs=2) as xopool,
            tc.sbuf_pool(name="h1pool", bufs=2) as h1pool,
            tc.sbuf_pool(name="h2pool", bufs=2) as h2pool,
            tc.sbuf_pool(name="ypool", bufs=2) as ypool,
            tc.sbuf_pool(name="spool", bufs=2) as spool,
            tc.psum_pool(name="psmm", bufs=3) as psmm,
            tc.psum_pool(name="psu", bufs=2) as psu,
            tc.psum_pool(name="pswb", bufs=2) as pswb,
            tc.psum_pool(name="psg", bufs=1) as psg,
        ):
            xTr = xT.rearrange("(kc p) b -> p kc b", p=P)
            xT8r = xT8.rearrange("(kc p) b -> p kc b", p=P)
            outr = outT.rearrange("(kc p) b -> p kc b", p=P)
            Vlr = Vl.rearrange("i (kc p) m -> p i kc m", p=P)
            Ulr = Ul.rearrange("i (mc p) d -> p i mc d", p=P)
            Cbr = Cb.rearrange("i m j p s -> p i m j s")

            def alloc_x0(c):
                t = xpool.tile([P, KC, CHUNK], WDT, tag="x0", name=f"x0_{c}")
                t8 = x8pool.tile([P, KC, CHUNK], F8, tag="x08", name=f"x08_{c}")
                return t, t8

            def load_x8(tt, c):
                cbs = slice(c * CHUNK, (c + 1) * CHUNK)
                nc.sync.dma_start(tt[1], xT8r[:, :, cbs])

            def load_xbf(tt, c):
                cbs = slice(c * CHUNK, (c + 1) * CHUNK)
                nc.sync.dma_start(tt[0], xTr[:, :, cbs])

            def load_x0(c):
                tt = alloc_x0(c)
                load_x8(tt, c)
                load_xbf(tt, c)
                return tt

            wg_sb = wpool.tile([P, KC, 2 * E], F8)
            nc.scalar.dma_start(wg_sb, WgT.rearrange("(kc p) e -> p kc e", p=P))

            v_sb = wpool.tile([P, N_CROSS, KC, ER], F8)
            u_sb = wpool.tile([P, N_CROSS, MC, D], F8)
            c_sb = wpool.tile([P, N_CROSS, MC, 2, P], F8)
            b_sb = wpool.tile([P, N_CROSS, KC], F32)
            id_sb = wpool.tile([P, P], WDT)

            # q1 (fast ramp) in need-order: x0c0 fp8 (gating), layer-0 V
            # (first big weight the PE blocks on), x0c1 fp8 (B gating),
            # U0; q10 takes the gating/C weights and the bf16 x copies
            # (first needed only at the combine).
            x0_tiles = {0: alloc_x0(0), 1: alloc_x0(1)}
            load_x8(x0_tiles[0], 0)
            nc.scalar.dma_start(c_sb[:, 0], Cbr[:, 0])
            nc.sync.dma_start(v_sb[:, 0, 0:KC // 2], Vlr[:, 0, 0:KC // 2])
            nc.sync.dma_start(v_sb[:, 0, KC // 2:], Vlr[:, 0, KC // 2:])
            nc.scalar.dma_start(
                x0_tiles[0][0], xTr[:, :, 0:CHUNK]
            )
            load_x8(x0_tiles[1], 1)
            nc.sync.dma_start(u_sb[:, 0], Ulr[:, 0])
            nc.sync.dma_start(id_sb, id128[:])
            sel_sb = wpool.tile([E, MC + 1, P], WDT)
            nc.scalar.dma_start(sel_sb, sel[:])
            nc.scalar.dma_start(c_sb[:, 1:], Cbr[:, 1:])
            nc.scalar.dma_start(v_sb[:, 1], Vlr[:, 1])
            load_xbf(x0_tiles[1], 1)
            nc.sync.dma_start(u_sb[:, 1], Ulr[:, 1])
            nc.scalar.dma_start(b_sb, bTc.rearrange("i p kc -> p i kc"))
            nc.scalar.dma_start(v_sb[:, 2], Vlr[:, 2])
            nc.scalar.dma_start(u_sb[:, 2], Ulr[:, 2])

            s_tiles = {}
            xi8_tiles = {}
            for pair in range(NCHUNK // 2):
                for i in range(N_CROSS):
                    for half in range(2):
                        c = 2 * pair + half
                        if i == 1 and c + 2 < NCHUNK:
                            x0_tiles[c + 2] = load_x0(c + 2)
                        bs = slice(c * CHUNK, (c + 1) * CHUNK)
                        x0, x08 = x0_tiles[c]
                        src8 = x08 if i == 0 else xi8_tiles.pop(c)
                        s_prev = None if i == 0 else s_tiles.pop(c)
                        last = i == N_CROSS - 1
                        if last:
                            xi = xopool.tile([P, KC, CHUNK], WDT, tag="xiout")
                            x0_tiles.pop(c)
                        else:
                            xi = xipool.tile([P, KC, CHUNK], F8, tag="xi8")
                            xi8_tiles[c] = xi
                            s_new = spool2.tile([P, KC, CHUNK], WDT, tag="s")
                            s_tiles[c] = s_new
                        # ---- gating (fp8 DoubleRow, 16-col padded) ----
                        gps = psg.tile([2 * E, CHUNK], F32, tag="g")
                        for q in range(KC // 2):
                            nc.tensor.matmul(
                                gps,
                                wg_sb[:, 2 * q : 2 * q + 2, :],
                                src8[:, 2 * q : 2 * q + 2, :],
                                start=(q == 0),
                                stop=(q == KC // 2 - 1),
                                perf_mode=DR,
                            )
                        expg = spool.tile([E, CHUNK], WDT, tag="expg")
                        nc.scalar.activation(expg, gps[0:E, :], AF.Exp)
                        # ---- V stage (fp8 DoubleRow) ----
                        h1 = h1pool.tile([P, MC, CHUNK], F8, tag="h1")
                        for mc in range(MC):
                            vps = psmm.tile([P, CHUNK], F32, tag="mm")
                            for q in range(KC // 2):
                                nc.tensor.matmul(
                                    vps,
                                    v_sb[:, i, 2 * q : 2 * q + 2,
                                         mc * P : (mc + 1) * P],
                                    src8[:, 2 * q : 2 * q + 2, :],
                                    start=(q == 0),
                                    stop=(q == KC // 2 - 1),
                                    perf_mode=DR,
                                )
                            nc.scalar.activation(h1[:, mc, :], vps, AF.Tanh)
                        # ---- softmax tail ----
                        sums = psg.tile([1, CHUNK], F32, tag="g")
                        nc.tensor.matmul(
                            sums, sel_sb[:, MC, 0:1], expg, start=True, stop=True
                        )
                        rfast = spool.tile([1, CHUNK], F32, tag="rfast")
                        nc.vector.reciprocal_approx_fast(rfast, sums)
                        rrow = spool.tile([1, CHUNK], WDT, tag="rrow")
                        nc.vector.tensor_copy(rrow, rfast)
                        wps = psg.tile([E, CHUNK], F32, tag="g")
                        nc.tensor.matmul(
                            wps, sel_sb[0:1, MC, 0:E], rrow, start=True, stop=True
                        )
                        wsb = spool.tile([E, CHUNK], WDT, tag="wsb")
                        nc.vector.tensor_tensor(wsb, expg, wps, OP.mult)
                        # ---- C stage (fp8 DoubleRow, zero-padded pair) ----
                        ys = ypool.tile([P, MC, CHUNK], F8, tag="y")
                        for mc in range(MC):
                            cps = psmm.tile([P, CHUNK], F32, tag="mm")
                            qb = (mc // 2) * 2
                            nc.tensor.matmul(
                                cps,
                                c_sb[:, i, mc, :, :],
                                h1[:, qb : qb + 2, :],
                                start=True,
                                stop=True,
                                perf_mode=DR,
                            )
                            wbp = pswb.tile([P, CHUNK], F32, tag="wb")
                            nc.tensor.matmul(
                                wbp, sel_sb[:, mc, :], wsb, start=True, stop=True
                            )
                            h2 = h2pool.tile([P, CHUNK], F32, tag="h2")
                            nc.scalar.activation(h2, cps, AF.Tanh)
                            nc.vector.tensor_tensor(ys[:, mc, :], h2, wbp, OP.mult)
                        # ---- U stage + S accumulate + combine ----
                        for dc in range(KC):
                            ups = psu.tile([P, CHUNK], F32, tag="u")
                            for q in range(MC // 2):
                                nc.tensor.matmul(
                                    ups,
                                    u_sb[:, i, 2 * q : 2 * q + 2,
                                         dc * P : (dc + 1) * P],
                                    ys[:, 2 * q : 2 * q + 2, :],
                                    start=(q == 0),
                                    stop=(s_prev is None and q == MC // 2 - 1),
                                    perf_mode=DR,
                                )
                            if s_prev is not None:
                                nc.tensor.matmul(
                                    ups, id_sb, s_prev[:, dc, :],
                                    start=False, stop=True,
                                )
                            # xi = (S + (1 + B_i)) * x0
                            nc.vector.scalar_tensor_tensor(
                                xi[:, dc, :],
                                ups,
                                b_sb[:, i, dc : dc + 1],
                                x0[:, dc, :],
                                OP.add,
                                OP.mult,
                            )
                            if not last:
                                nc.scalar.activation(
                                    s_new[:, dc, :], ups, AF.Copy
                                )
                            else:
                                eng = nc.sync if dc % 2 == 0 else nc.scalar
                                eng.dma_start(outr[:, dc, bs], xi[:, dc, :])
    nc.compile()
    return nc


_CTX = {}


def _get_nc():
    if "nc" not in _CTX:
        _CTX["nc"] = _build()
    return _CTX["nc"]


def _prep_weights(U, V, C, Wg, b):
    f = np.float32
    U = np.asarray(U, dtype=f)
    V = np.asarray(V, dtype=f)
    C = np.asarray(C, dtype=f)
    Wg = np.asarray(Wg, dtype=f)
    b = np.asarray(b, dtype=f)
    # Vl[i, d, e*R+r] = V[i, e, d, r]
    Vl = np.ascontiguousarray(V.transpose(0, 2, 1, 3).reshape(N_CROSS, D, ER))
    # Ul[i, e*R+r, d] = U[i, e, d, r]
    Ul = np.ascontiguousarray(U.transpose(0, 1, 3, 2).reshape(N_CROSS, ER, D))
    # DoubleRow C: out-block mc pairs rhs h1 blocks (qb, qb+1); the plane
    # matching block mc carries the block-diag expert pair, the other is 0.
    Cb2 = np.zeros((N_CROSS, MC, 2, P, P), dtype=f)
    for i in range(N_CROSS):
        for m in range(MC):
            blk = np.zeros((P, P), dtype=f)
            blk[:R, :R] = C[i, 2 * m]
            blk[R:, R:] = C[i, 2 * m + 1]
            Cb2[i, m, m % 2] = blk
    WgT = np.zeros((D, 2 * E), dtype=f)
    WgT[:, :E] = Wg.T
    # bTc[i, p, kc] = 1 + cumsum_i b  (stt per-partition scalar)
    bc = 1.0 + np.cumsum(b, axis=0)
    bTc = np.ascontiguousarray(bc.reshape(N_CROSS, KC, P).transpose(0, 2, 1))
    sel = np.zeros((E, MC + 1, P), dtype=f)
    for m in range(MC):
        for j in range(P):
            sel[2 * m + j // R, m, j] = 1.0
    sel[:, MC, :] = 1.0
    return dict(
        Vl=Vl.astype(NPF8),
        Ul=Ul.astype(NPF8),
        Cb=Cb2.astype(NPF8),
        WgT=WgT.astype(NPF8),
        bTc=bTc,
        sel=sel.astype(BF16),
        id128=np.eye(P, dtype=f).astype(BF16),
    )


def kernel(x, U, V, C, Wg, b, _trace=False):
    nc = _get_nc()
    w = _prep_weights(U, V, C, Wg, b)
    xs = np.asarray(x, dtype=np.float32).reshape(NCORES, BC, D)
    in_maps = []
    for ci in range(NCORES):
        xt = np.ascontiguousarray(xs[ci].T)
        m = {"xT": xt.astype(BF16), "xT8": xt.astype(NPF8)}
        m.update(w)
        in_maps.append(m)
    res = run_bass_kernel_spmd(nc, in_maps, list(range(NCORES)), trace=_trace)
    kernel.last_result = res
    out = np.concatenate(
        [np.asarray(res.results[ci]["outT"]).astype(np.float32).T
         for ci in range(NCORES)],
        axis=0,
    )
    return np.ascontiguousarray(out, dtype=np.float32)
